# revision 29
# baseline (speedup 1.0000x reference)
"""Trainium2 Bass kernel for nn_Downstream_38439957299924 (gnn_message_passing).

SPMD over 8 NeuronCores, 1D node partition (1024 rows/core).

  fea   = elu(x * wcomb)                          wcomb = cw00*prompt + cw01*shared
  agg   = Anorm @ fea                             Anorm built dense on host from edges
  h     = concat(fea, agg) * balance_tok ; hn = h / (||h|| + eps)
  sims  = hn @ hn.T ; t_i = 17th largest of row i  (K+1 = 17 with self edge)
  Wsym  = relu(sims * (sims >= min(t_i, t_j)))    == to_undirected(mean)+relu of ref
  A_tot = alpha*Anorm + (1-alpha)*Wsym
  h1    = relu((A_tot @ fea) @ W1) ; emb = (A_tot @ h1) @ W2
  out   = cos(emb[node_idx], class prototypes) / TEMP

v2 structure (vs the f32r/DRAM-spill v1):
- Everything on the fat path is fp16: anorm, fea, hnT, sims, A_tot, h1, emb.
  fp16 rhs streams 1 col/cycle through the PE (fp32 streams at half rate),
  halving matmul time; thresholds stay fp32/fp16-exact so the mask is
  bitwise consistent with the stored sims.
- simsT tiles [128 j, 1024 i] live in SBUF (64 tiles, 128KB/partition) and
  are overwritten in place by the A_totT tiles: no DRAM spill at all
  (v1 moved 96MB/core of simsT+atotT through HBM).
- relu is folded into the threshold: t' = max(t, 0), mask = sims >= min(t'_i,t'_j).
- Q = A_tot @ fea and P = A_tot @ h1 are computed TRANSPOSED (stationary =
  fea/h1 slices, moving = A_totT tiles), so QT/PT drop out in exactly the
  layout the W1/W2 matmuls need as lhsT — no interstage PE transposes.
- Per-row top-k candidates from PE-transposed copies of the fp16 sims tiles
  (max8 per 128-node window, direct from PSUM), then 3-round max/match_replace.
- DMA is quad-batched (1MB anorm/h1/fea loads).
- Collectives: AllGather fea16/hnT/t/h1/emb (all fp16) + AllReduce class sums.
"""
import numpy as np

import concourse.bacc as bacc
import concourse.bass as bass
import concourse.mybir as mybir
import concourse.tile as tile
from concourse.bass_utils import run_bass_kernel_spmd
from concourse.masks import make_identity

FP = mybir.dt.float32
HF = mybir.dt.float16
AF = mybir.ActivationFunctionType
ALU = mybir.AluOpType

NCORES = 8
N = 8192          # nodes
F = 256           # input feature dim
H2 = 512          # concat feature dim
HID = 256         # gnn hidden dim
NCLS = 10
NSEL = 4096
TEMP = 0.2
EPS = 1e-8
P = 128

NEGINF = -3.0e38


def build_program(alpha: float, n=N, ncores=NCORES, nsel=NSEL,
                  debug_outputs=False):
    """Emit the SPMD Bass program for one core (SPMD across ncores)."""
    R = n // ncores           # rows per core
    NB = n // P               # global node blocks of 128
    LB = R // P               # local node blocks
    KB = H2 // P              # feature k-blocks (4)
    FB = F // P               # 256-dim k-blocks (2)
    JC = R // 512             # 512-wide j chunks per rank
    SELC = nsel // ncores     # selected nodes per core
    SB = SELC // P            # selected blocks per core
    one_m_alpha = 1.0 - alpha
    use_anorm = alpha > 1e-6
    use_wsym = one_m_alpha > 1e-6
    # host ships anorm16 * (alpha/post_scale); Q/P evicts multiply by
    # post_scale, so the C2 merge is a plain add with no per-tile scaling.
    post_scale = one_m_alpha if use_wsym else 1.0
    # anorm16 is always needed for agg; alpha folding only when it feeds A_tot
    an_ship = (alpha / post_scale) if use_anorm else 1.0
    agg_scale = 1.0 / an_ship
    assert R % 512 == 0 and SELC % P == 0 and NB % 4 == 0

    nc = bacc.Bacc(None)

    # ---- per-core external inputs ----
    x_l = nc.declare_dram_parameter("x_l", [R, F], FP, isOutput=False)
    wcomb = nc.declare_dram_parameter("wcomb", [1, F], FP, isOutput=False)
    baltok = nc.declare_dram_parameter("baltok", [1, H2], FP, isOutput=False)
    anorm16 = nc.declare_dram_parameter("anorm16", [n, R], HF, isOutput=False)
    w116 = nc.declare_dram_parameter("w116", [F, HID], HF, isOutput=False)
    w216 = nc.declare_dram_parameter("w216", [HID, HID], HF, isOutput=False)
    selidx = nc.declare_dram_parameter("selidx", [P, SB], mybir.dt.int32, isOutput=False)
    # selw pre-transposed on host to [P, LB*NCLS] (partition-major)
    selw = nc.declare_dram_parameter("selw", [P, LB * NCLS], HF, isOutput=False)
    out = nc.declare_dram_parameter("out", [SELC, NCLS], FP, isOutput=True)
    if debug_outputs:
        dbg_t = nc.declare_dram_parameter("dbg_t", [R, 1], HF, isOutput=True)
        dbg_agg = nc.declare_dram_parameter("dbg_agg", [F, R], FP, isOutput=True)
        dbg_h1 = nc.declare_dram_parameter("dbg_h1", [R, HID], HF, isOutput=True)
        dbg_emb = nc.declare_dram_parameter("dbg_emb", [R, HID], HF, isOutput=True)
        dbg_hnT = nc.declare_dram_parameter("dbg_hnT", [H2, R], HF, isOutput=True)

    # ---- internal DRAM ----
    fea16_bounce = nc.dram_tensor("fea16_bounce", [R, F], HF)
    fea16_full = nc.dram_tensor("fea16_full", [n, F], HF, addr_space="Shared")
    # hnT split into JC column-halves so each half's AllGather can fire as
    # soon as the corresponding local i-blocks are normalized
    hnT_bounce = [nc.dram_tensor(f"hnT_bounce{j}", [H2, 512], HF)
                  for j in range(JC)]
    hnT_all = [nc.dram_tensor(f"hnT_all{j}", [ncores * H2, 512], HF,
                              addr_space="Shared") for j in range(JC)]
    t_bounce = nc.dram_tensor("t_bounce", [R, 1], HF)
    t_full = nc.dram_tensor("t_full", [n, 1], HF, addr_space="Shared")
    h1_bounce = nc.dram_tensor("h1_bounce", [R, HID], HF)
    h1_full = nc.dram_tensor("h1_full", [n, HID], HF, addr_space="Shared")
    emb_bounce = nc.dram_tensor("emb_bounce", [R, HID], HF)
    emb_full = nc.dram_tensor("emb_full", [n, HID], HF, addr_space="Shared")
    sums_bounce = nc.dram_tensor("sums_bounce", [HID, NCLS], FP)
    sums_red = nc.dram_tensor("sums_red", [HID, NCLS], FP, addr_space="Shared")

    rg = [list(range(ncores))]

    def ag(in_ap, out_ap):
        nc.gpsimd.collective_compute(
            "AllGather", ALU.bypass, replica_groups=rg, ins=[in_ap], outs=[out_ap])

    with tile.TileContext(nc) as tc:
        with (
            tc.tile_pool(name="const", bufs=1) as const,
            tc.tile_pool(name="live", bufs=1) as live,
        ):
            ident32 = const.tile([P, P], FP)
            make_identity(nc, ident32)
            ident16 = const.tile([P, P], HF)
            nc.scalar.activation(ident16[:], ident32[:], AF.Copy)
            hnT_l = [live.tile([P, R], HF, tag=f"hnT{k}", name=f"hnT{k}")
                     for k in range(KB)]
            emball = live.tile([P, LB * HID], HF, tag="emball", name="emball")
            # the big one: simsT (later overwritten in place by A_totT)
            sims_sb = [live.tile([P, R], HF, tag=f"sims{m}", name=f"sims{m}")
                       for m in range(NB)]

            # ===== phase 0: fea_l = elu(x_l * wcomb); all-gather fea16 =====
            with (
                tc.tile_pool(name="pfea", bufs=1) as pfea,
                tc.tile_pool(name="p0c", bufs=1) as p0c,
                tc.tile_pool(name="p0", bufs=3) as p0,
            ):
                fea_l = [pfea.tile([P, F], FP, tag=f"fea{b}", name=f"fea{b}")
                         for b in range(LB)]
                wcomb_b = p0c.tile([P, F], FP)
                nc.sync.dma_start(wcomb_b[:], wcomb[:1, :].to_broadcast([P, F]))
                baltok_b = p0c.tile([P, H2], FP)
                nc.sync.dma_start(baltok_b[:], baltok[:1, :].to_broadcast([P, H2]))
                for b in range(LB):
                    xt = p0.tile([P, F], FP, tag="xt")
                    nc.sync.dma_start(xt[:], x_l[b * P:(b + 1) * P, :])
                    z = p0.tile([P, F], FP, tag="z")
                    nc.vector.tensor_mul(z[:], xt[:], wcomb_b[:])
                    r = p0.tile([P, F], FP, tag="r")
                    nc.scalar.activation(r[:], z[:], AF.Relu)
                    m0 = p0.tile([P, F], FP, tag="m0")
                    nc.vector.tensor_scalar_min(m0[:], z[:], 0.0)
                    e = p0.tile([P, F], FP, tag="e")
                    nc.scalar.activation(e[:], m0[:], AF.Exp)
                    nc.vector.scalar_tensor_tensor(
                        fea_l[b][:], e[:], -1.0, r[:], op0=ALU.add, op1=ALU.add)
                    f16 = p0.tile([P, F], HF, tag="f16")
                    nc.scalar.activation(f16[:], fea_l[b][:], AF.Copy)
                    nc.sync.dma_start(fea16_bounce[b * P:(b + 1) * P, :], f16[:])
                ag(fea16_bounce[:], fea16_full[:])

                # ===== phase A: aggT[f, i] = sum_j fea16[j, f] * anorm16[j, i] =====
                with tc.tile_pool(name="pagg", bufs=1) as pagg:
                    aggT_sb = [pagg.tile([P, R], FP, tag=f"aggT{mf}", name=f"aggT{mf}")
                               for mf in range(FB)]
                    with (
                        tc.tile_pool(name="pa_in", bufs=3) as pa_in,
                        tc.tile_pool(name="pa_ps", bufs=1, space="PSUM") as pa_ps,
                    ):
                        agg_ps = [[pa_ps.tile([P, 512], FP, tag=f"aps{mf}_{c2}",
                                              name=f"aps{mf}_{c2}")
                                   for c2 in range(JC)] for mf in range(FB)]
                        for kq in range(NB // 4):
                            fk4 = pa_in.tile([P, 4 * F], HF, tag="fk4")
                            nc.scalar.dma_start(
                                fk4[:].rearrange("p (four f) -> p four f", four=4),
                                fea16_full[kq * 4 * P:(kq + 1) * 4 * P, :].rearrange(
                                    "(four p) f -> p four f", four=4))
                            an4 = pa_in.tile([P, 4 * R], HF, tag="an4")
                            nc.sync.dma_start(
                                an4[:].rearrange("p (four i) -> p four i", four=4),
                                anorm16[kq * 4 * P:(kq + 1) * 4 * P, :].rearrange(
                                    "(four p) i -> p four i", four=4))
                            for q4 in range(4):
                                kj = kq * 4 + q4
                                for mf in range(FB):
                                    for c2 in range(JC):
                                        nc.tensor.matmul(
                                            agg_ps[mf][c2][:],
                                            fk4[:, q4 * F + mf * P:q4 * F + (mf + 1) * P],
                                            an4[:, q4 * R + c2 * 512:q4 * R + (c2 + 1) * 512],
                                            start=(kj == 0), stop=(kj == NB - 1))
                        for mf in range(FB):
                            for c2 in range(JC):
                                nc.scalar.activation(
                                    aggT_sb[mf][:, c2 * 512:(c2 + 1) * 512],
                                    agg_ps[mf][c2][:], AF.Copy, scale=agg_scale)
                        if debug_outputs:
                            for mf in range(FB):
                                nc.sync.dma_start(
                                    dbg_agg[mf * P:(mf + 1) * P, :], aggT_sb[mf][:])

                    # ===== phase A2: h = concat(fea, aggT.T)*baltok; hn; hnT =====
                    with (
                        tc.tile_pool(name="ph_ps", bufs=2, space="PSUM") as ph_ps,
                        tc.tile_pool(name="ph_sb", bufs=2) as ph_sb,
                    ):
                        for b in range(LB):
                            h = ph_sb.tile([P, H2], FP, tag="h")
                            nc.vector.tensor_mul(h[:, :F], fea_l[b][:], baltok_b[:, :F])
                            for mf in range(FB):
                                tp = ph_ps.tile([P, P], FP, tag="tp")
                                nc.tensor.transpose(
                                    tp[:], aggT_sb[mf][:, b * P:(b + 1) * P], ident32[:])
                                nc.scalar.activation(
                                    h[:, F + mf * P:F + (mf + 1) * P], tp[:], AF.Copy)
                            nc.vector.tensor_mul(h[:, F:], h[:, F:], baltok_b[:, F:])
                            sq = ph_sb.tile([P, H2], FP, tag="sq")
                            ssq = ph_sb.tile([P, 1], FP, tag="ssq")
                            nc.scalar.activation(sq[:], h[:], AF.Square, accum_out=ssq[:])
                            nrm = ph_sb.tile([P, 1], FP, tag="nrm")
                            nc.scalar.activation(nrm[:], ssq[:], AF.Sqrt)
                            nc.vector.tensor_scalar_add(nrm[:], nrm[:], EPS)
                            inv = ph_sb.tile([P, 1], FP, tag="inv")
                            nc.vector.reciprocal(inv[:], nrm[:])
                            hn = ph_sb.tile([P, H2], FP, tag="hn")
                            nc.vector.tensor_scalar(hn[:], h[:], inv[:, :1], None, ALU.mult)
                            for kk in range(KB):
                                tp2 = ph_ps.tile([P, P], FP, tag="tp2")
                                nc.tensor.transpose(
                                    tp2[:], hn[:, kk * P:(kk + 1) * P], ident32[:])
                                nc.scalar.activation(
                                    hnT_l[kk][:, b * P:(b + 1) * P], tp2[:], AF.Copy)
                            # fire each 512-column half's gather as soon as done
                            if (b + 1) % 4 == 0:
                                jh = b // 4
                                for kk in range(KB):
                                    nc.sync.dma_start(
                                        hnT_bounce[jh][kk * P:(kk + 1) * P, :],
                                        hnT_l[kk][:, jh * 512:(jh + 1) * 512])
                                ag(hnT_bounce[jh][:], hnT_all[jh][:])

            # ===== phase C1: simsT tiles in SBUF (fp16); candidates from
            #       PE-transposed tiles batched 4-wide in PSUM (max8 per
            #       512-node window) =====
            with tc.tile_pool(name="pb_cand", bufs=1) as pb_cand:
                cand = [pb_cand.tile([P, 2 * NB], FP, tag=f"cand{m8}", name=f"cand{m8}")
                        for m8 in range(LB)]
                with (
                    tc.tile_pool(name="pc1_in", bufs=3) as pc1_in,
                    tc.tile_pool(name="pc1_ps", bufs=2, space="PSUM") as pc1_ps,
                    tc.tile_pool(name="pc1_tp", bufs=1, space="PSUM") as pc1_tp,
                ):
                    # tp_acc: per local i-block, a [128, 512] fp16 window
                    # accumulating 4 mj transposes; two i-blocks share a bank
                    tp_acc = [pc1_tp.tile([P, 1024], HF, tag=f"tpa{q}",
                                          name=f"tpa{q}") for q in range(LB // 2)]
                    for jc in range(JC):
                        for r0 in range(ncores):
                            lhs = pc1_in.tile([P, KB * 512], HF, tag="lhs")
                            nc.sync.dma_start(
                                lhs[:].rearrange("p (kk j) -> p kk j", kk=KB),
                                hnT_all[jc][r0 * H2:(r0 + 1) * H2, :].rearrange(
                                    "(kk p) j -> p kk j", kk=KB))
                            mq = r0 * JC + jc
                            for sub in range(4):
                                mj = mq * 4 + sub
                                for hh in range(JC):
                                    ps = pc1_ps.tile([P, 512], FP, tag="ps")
                                    for kk in range(KB):
                                        nc.tensor.matmul(
                                            ps[:],
                                            lhs[:, kk * 512 + sub * P:kk * 512 + (sub + 1) * P],
                                            hnT_l[kk][:, hh * 512:(hh + 1) * 512],
                                            start=(kk == 0), stop=(kk == KB - 1))
                                    nc.scalar.activation(
                                        sims_sb[mj][:, hh * 512:(hh + 1) * 512],
                                        ps[:], AF.Copy)
                                for ib8 in range(LB):
                                    nc.tensor.transpose(
                                        tp_acc[ib8 // 2][
                                            :, (ib8 % 2) * 512 + sub * P:
                                            (ib8 % 2) * 512 + (sub + 1) * P],
                                        sims_sb[mj][:, ib8 * P:(ib8 + 1) * P],
                                        ident16[:])
                            for ib8 in range(LB):
                                nc.vector.max(
                                    cand[ib8][:, mq * 8:(mq + 1) * 8],
                                    tp_acc[ib8 // 2][
                                        :, (ib8 % 2) * 512:(ib8 % 2 + 1) * 512])
                # merge candidates -> t' = max(17th largest, 0) per local row
                with tc.tile_pool(name="pbm_sb", bufs=2) as pbm_sb:
                    for m8 in range(LB):
                        t8a = pbm_sb.tile([P, 8], FP, tag="t8a")
                        nc.vector.max(t8a[:], cand[m8][:])
                        nc.vector.match_replace(
                            cand[m8][:], t8a[:], cand[m8][:], NEGINF)
                        t8b = pbm_sb.tile([P, 8], FP, tag="t8b")
                        nc.vector.max(t8b[:], cand[m8][:])
                        nc.vector.match_replace(
                            cand[m8][:], t8b[:], cand[m8][:], NEGINF)
                        t8c = pbm_sb.tile([P, 8], FP, tag="t8c")
                        nc.vector.max(t8c[:], cand[m8][:])
                        tcl = pbm_sb.tile([P, 1], HF, tag="tcl")
                        nc.vector.tensor_scalar(
                            tcl[:], t8c[:, :1], 0.0, None, ALU.max)
                        nc.sync.dma_start(
                            t_bounce[m8 * P:(m8 + 1) * P, :], tcl[:])
            ag(t_bounce[:], t_full[:])

            # ===== phase C2: A_totT = alpha*anorm + (1-alpha)*masked sims,
            #       written in place over sims_sb; fused QT = (A_tot @ fea).T =====
            with (
                tc.tile_pool(name="pqt", bufs=1) as pqt,
                tc.tile_pool(name="pc_tib", bufs=1) as pc_tib,
                tc.tile_pool(name="pc_in", bufs=2) as pc_in,
                tc.tile_pool(name="pc_qps", bufs=1, space="PSUM") as pc_qps,
                tc.tile_pool(name="pc_sb", bufs=3) as pc_sb,
            ):
                qT16 = [pqt.tile([P, R], HF, tag=f"qT{fs}", name=f"qT{fs}")
                        for fs in range(FB)]
                t_i16 = pc_tib.tile([P, R], HF)
                nc.sync.dma_start(
                    t_i16[:],
                    t_bounce.rearrange("a b -> b a")[:1, :].to_broadcast([P, R]))
                # t_full as [128, NB]: column mj = t' for node block mj. Loaded
                # block-per-partition (contiguous 256B lines) then PE-transposed
                # — the direct [p, m] gather would be a 2-byte-strided DMA.
                # tfneg = -(t_j - half ulp16): the Sign-activation bias, shifted
                # off the fp16 grid so sign(st + tfneg) > 0  <=>  st >= t_j.
                tfL = pc_tib.tile([NB, P], HF)
                nc.sync.dma_start(
                    tfL[:NB, :], t_full.rearrange("(m p) one -> m (p one)", p=P))
                tf_sb = pc_tib.tile([P, NB], HF)
                with tc.tile_pool(name="ptf", bufs=1, space="PSUM") as ptf:
                    tfp = ptf.tile([P, NB], HF)
                    nc.tensor.transpose(
                        tfp[:, :], tfL[:NB, :], ident16[:NB, :NB])
                    nc.scalar.activation(tf_sb[:], tfp[:], AF.Copy)
                tfneg = pc_tib.tile([P, NB], FP)
                nc.vector.tensor_scalar(
                    tfneg[:], tf_sb[:], -(1.0 - 2.0 ** -12), 1e-8,
                    ALU.mult, ALU.add)
                bias30 = pc_tib.tile([P, 1], FP)
                nc.vector.memset(bias30[:], 30.0)
                q_ps = [[pc_qps.tile([P, 512], FP, tag=f"q{fs}_{ih}",
                                     name=f"q{fs}_{ih}")
                         for ih in range(JC)] for fs in range(FB)]
                for mq in range(NB // 4):
                    an4 = pc_in.tile([P, 4 * R], HF, tag="an4")
                    if use_anorm:
                        nc.sync.dma_start(
                            an4[:].rearrange("p (four i) -> p four i", four=4),
                            anorm16[mq * 4 * P:(mq + 1) * 4 * P, :].rearrange(
                                "(four p) i -> p four i", four=4))
                    fq4 = pc_in.tile([P, 4 * F], HF, tag="fq4")
                    nc.scalar.dma_start(
                        fq4[:].rearrange("p (four f) -> p four f", four=4),
                        fea16_full[mq * 4 * P:(mq + 1) * 4 * P, :].rearrange(
                            "(four p) f -> p four f", four=4))
                    for q4 in range(4):
                        mj = mq * 4 + q4
                        st = sims_sb[mj]
                        if use_wsym:
                            # mask = (st >= t_i) OR (st >= t_j), thresholds
                            # clamped >= 0 (relu folded in). is_ge has no fast
                            # DVE path, so the mask is built comparison-free:
                            #   sj = sign(st - t_j')            in {-1,+1}
                            #   q  = max(st - t_i, sj)          >= 0 iff mask
                            #   g  = sigmoid(1e9*q + 30)        exactly {0,1}
                            # (q is an exact-sign fp16: nonzero |q| >= 2^-24,
                            # so 1e9*q +- 30 saturates the sigmoid both ways)
                            sj = pc_sb.tile([P, R], HF, tag="sj")
                            nc.scalar.activation(
                                sj[:], st[:], AF.Sign, bias=tfneg[:, mj:mj + 1])
                            d = pc_sb.tile([P, R], HF, tag="d")
                            nc.vector.tensor_tensor(
                                d[:], st[:], t_i16[:], ALU.subtract)
                            q = pc_sb.tile([P, R], HF, tag="q")
                            nc.vector.tensor_tensor(d[:], d[:], sj[:], ALU.max)
                            nc.scalar.activation(
                                q[:], d[:], AF.Sigmoid, bias=bias30[:, :1],
                                scale=1e9)
                            eng = nc.gpsimd if mj % 3 == 1 else nc.vector
                            if use_anorm:
                                w2 = pc_sb.tile([P, R], HF, tag="w2")
                                eng.tensor_tensor(w2[:], st[:], q[:], ALU.mult)
                                eng.tensor_tensor(
                                    st[:], w2[:], an4[:, q4 * R:(q4 + 1) * R],
                                    ALU.add)
                            else:
                                eng.tensor_tensor(st[:], st[:], q[:], ALU.mult)
                        else:
                            nc.vector.tensor_copy(
                                st[:], an4[:, q4 * R:(q4 + 1) * R])
                        # QT[f, i] += fea16[j, f].T @ at[j, i]
                        for fs in range(FB):
                            for ih in range(JC):
                                nc.tensor.matmul(
                                    q_ps[fs][ih][:],
                                    fq4[:, q4 * F + fs * P:q4 * F + (fs + 1) * P],
                                    st[:, ih * 512:(ih + 1) * 512],
                                    start=(mj == 0), stop=(mj == NB - 1))
                for fs in range(FB):
                    for ih in range(JC):
                        nc.scalar.activation(
                            qT16[fs][:, ih * 512:(ih + 1) * 512],
                            q_ps[fs][ih][:], AF.Copy, scale=post_scale)

                # ===== phase D1: h1 = relu(QT.T @ W1); all-gather h1 =====
                with (
                    tc.tile_pool(name="pd1_ps", bufs=2, space="PSUM") as pd1_ps,
                    tc.tile_pool(name="pd1_sb", bufs=2) as pd1_sb,
                    tc.tile_pool(name="pd1_w", bufs=1) as pd1_w,
                ):
                    w1_sb = [pd1_w.tile([P, HID], HF, tag=f"w1_{k2}", name=f"w1_{k2}")
                             for k2 in range(FB)]
                    for k2 in range(FB):
                        nc.sync.dma_start(w1_sb[k2][:], w116[k2 * P:(k2 + 1) * P, :])
                    h1all = pd1_sb.tile([P, LB * HID], HF)
                    for m8 in range(LB):
                        ps = pd1_ps.tile([P, HID], FP, tag="psh")
                        for k2 in range(FB):
                            nc.tensor.matmul(
                                ps[:], qT16[k2][:, m8 * P:(m8 + 1) * P], w1_sb[k2][:],
                                start=(k2 == 0), stop=(k2 == FB - 1))
                        nc.scalar.activation(
                            h1all[:, m8 * HID:(m8 + 1) * HID], ps[:], AF.Relu)
                    nc.sync.dma_start(
                        h1_bounce.rearrange("(m p) e -> p m e", p=P),
                        h1all[:].rearrange("p (m e) -> p m e", m=LB))
            ag(h1_bounce[:], h1_full[:])

            # ===== phase D2: PT = (A_tot @ h1_full).T =====
            with (
                tc.tile_pool(name="ppt", bufs=1) as ppt,
                tc.tile_pool(name="pd2_in", bufs=3) as pd2_in,
                tc.tile_pool(name="pd2_ps", bufs=1, space="PSUM") as pd2_ps,
            ):
                pT16 = [ppt.tile([P, R], HF, tag=f"pT{hs}", name=f"pT{hs}")
                        for hs in range(FB)]
                p_ps = [[pd2_ps.tile([P, 512], FP, tag=f"p{hs}_{ih}",
                                     name=f"p{hs}_{ih}")
                         for ih in range(JC)] for hs in range(FB)]
                for kq in range(NB // 4):
                    h1k4 = pd2_in.tile([P, 4 * HID], HF, tag="h1k4")
                    nc.sync.dma_start(
                        h1k4[:].rearrange("p (four e) -> p four e", four=4),
                        h1_full[kq * 4 * P:(kq + 1) * 4 * P, :].rearrange(
                            "(four p) e -> p four e", four=4))
                    for q4 in range(4):
                        kj = kq * 4 + q4
                        for hs in range(FB):
                            for ih in range(JC):
                                nc.tensor.matmul(
                                    p_ps[hs][ih][:],
                                    h1k4[:, q4 * HID + hs * P:q4 * HID + (hs + 1) * P],
                                    sims_sb[kj][:, ih * 512:(ih + 1) * 512],
                                    start=(kj == 0), stop=(kj == NB - 1))
                for hs in range(FB):
                    for ih in range(JC):
                        nc.scalar.activation(
                            pT16[hs][:, ih * 512:(ih + 1) * 512],
                            p_ps[hs][ih][:], AF.Copy, scale=post_scale)

                # ===== phase D3: emb = PT.T @ W2; all-gather emb =====
                with (
                    tc.tile_pool(name="pd3_ps", bufs=2, space="PSUM") as pd3_ps,
                    tc.tile_pool(name="pd3_w", bufs=1) as pd3_w,
                    tc.tile_pool(name="pd3_sb", bufs=2) as pd3_sb,
                ):
                    w2_sb = [pd3_w.tile([P, HID], HF, tag=f"w2_{k2}", name=f"w2_{k2}")
                             for k2 in range(FB)]
                    for k2 in range(FB):
                        nc.sync.dma_start(w2_sb[k2][:], w216[k2 * P:(k2 + 1) * P, :])
                    for m8 in range(LB):
                        ps = pd3_ps.tile([P, HID], FP, tag="pse")
                        for k2 in range(FB):
                            nc.tensor.matmul(
                                ps[:], pT16[k2][:, m8 * P:(m8 + 1) * P], w2_sb[k2][:],
                                start=(k2 == 0), stop=(k2 == FB - 1))
                        nc.scalar.activation(
                            emball[:, m8 * HID:(m8 + 1) * HID], ps[:], AF.Copy)
                    nc.sync.dma_start(
                        emb_bounce.rearrange("(m p) e -> p m e", p=P),
                        emball[:].rearrange("p (m e) -> p m e", m=LB))
            ag(emb_bounce[:], emb_full[:])

            # ===== phase E: prototypes + cosine scores =====
            with (
                tc.tile_pool(name="pe_sb", bufs=1) as pe_sb,
                tc.tile_pool(name="pe_ps", bufs=1, space="PSUM") as pe_ps,
                tc.tile_pool(name="pe_sc", bufs=2) as pe_sc,
            ):
                # class sums from LOCAL emb rows via the host-built per-node
                # count matrix selw (runs concurrently with the emb AllGather)
                selw_sb = pe_sb.tile([P, LB * NCLS], HF)
                nc.sync.dma_start(selw_sb[:], selw[:])
                sums_ps = [pe_ps.tile([P, NCLS], FP, tag=f"sums{b2}", name=f"sums{b2}")
                           for b2 in range(FB)]
                for m8 in range(LB):
                    for b2 in range(FB):
                        nc.tensor.matmul(
                            sums_ps[b2][:],
                            emball[:, m8 * HID + b2 * P:m8 * HID + (b2 + 1) * P],
                            selw_sb[:, m8 * NCLS:(m8 + 1) * NCLS],
                            start=(m8 == 0), stop=(m8 == LB - 1))
                for b2 in range(FB):
                    st = pe_sc.tile([P, NCLS], FP, tag="st")
                    nc.scalar.activation(st[:], sums_ps[b2][:], AF.Copy)
                    nc.sync.dma_start(sums_bounce[b2 * P:(b2 + 1) * P, :], st[:])
                nc.gpsimd.collective_compute(
                    "AllReduce", ALU.add, replica_groups=rg,
                    ins=[sums_bounce[:]], outs=[sums_red[:]])
                idx_sb = pe_sb.tile([P, SB], mybir.dt.int32)
                nc.sync.dma_start(idx_sb[:], selidx[:])
                sel_sb = [pe_sb.tile([P, HID], HF, tag=f"sel{q}", name=f"sel{q}")
                          for q in range(SB)]
                sc_q = [pe_sb.tile([P, 1], FP, tag=f"scq{q}", name=f"scq{q}")
                        for q in range(SB)]
                for q in range(SB):
                    nc.gpsimd.indirect_dma_start(
                        out=sel_sb[q][:], out_offset=None,
                        in_=emb_full[:],
                        in_offset=bass.IndirectOffsetOnAxis(
                            ap=idx_sb[:, q:q + 1], axis=0))
                    sq = pe_sc.tile([P, HID], FP, tag="sq")
                    ssq = pe_sc.tile([P, 1], FP, tag="ssq")
                    nc.scalar.activation(
                        sq[:], sel_sb[q][:], AF.Square, accum_out=ssq[:])
                    nrm = pe_sc.tile([P, 1], FP, tag="nrm2")
                    nc.scalar.activation(nrm[:], ssq[:], AF.Sqrt)
                    nc.vector.tensor_scalar_add(nrm[:], nrm[:], EPS)
                    nc.vector.tensor_scalar_mul(nrm[:], nrm[:], TEMP)
                    nc.vector.reciprocal(sc_q[q][:], nrm[:])
                sums_sb = [pe_sb.tile([P, NCLS], FP, tag=f"smr{b2}", name=f"smr{b2}")
                           for b2 in range(FB)]
                ones_col = pe_sb.tile([P, 1], FP)
                nc.vector.memset(ones_col[:], 1.0)
                ones_row = pe_sb.tile([1, P], FP)
                nc.vector.memset(ones_row[:1, :], 1.0)
                nps = pe_ps.tile([1, NCLS], FP, tag="nps")
                for b2 in range(FB):
                    nc.sync.dma_start(sums_sb[b2][:], sums_red[b2 * P:(b2 + 1) * P, :])
                    sqs = pe_sc.tile([P, NCLS], FP, tag="sqs")
                    nc.scalar.activation(sqs[:], sums_sb[b2][:], AF.Square)
                    nc.tensor.matmul(nps[:1, :], ones_col[:, :1], sqs[:],
                                     start=(b2 == 0), stop=(b2 == FB - 1))
                nrmc = pe_sc.tile([1, NCLS], FP, tag="nrmc")
                nc.scalar.activation(nrmc[:1, :], nps[:1, :], AF.Sqrt)
                nc.vector.tensor_scalar_add(nrmc[:1, :], nrmc[:1, :], EPS)
                invc = pe_sc.tile([1, NCLS], FP, tag="invc")
                nc.vector.reciprocal(invc[:1, :], nrmc[:1, :])
                bcp = pe_ps.tile([P, NCLS], FP, tag="bcp")
                nc.tensor.matmul(bcp[:], ones_row[:1, :], invc[:1, :],
                                 start=True, stop=True)
                bc_sb = pe_sb.tile([P, NCLS], FP)
                nc.scalar.activation(bc_sb[:], bcp[:], AF.Copy)
                pnT = [pe_sb.tile([P, NCLS], FP, tag=f"pnT{b2}", name=f"pnT{b2}")
                       for b2 in range(FB)]
                for b2 in range(FB):
                    nc.vector.tensor_mul(pnT[b2][:], sums_sb[b2][:], bc_sb[:])
                selT = [pe_sb.tile([P, SELC], FP, tag=f"selT{b2}", name=f"selT{b2}")
                        for b2 in range(FB)]
                for q in range(SB):
                    for b2 in range(FB):
                        tp = pe_ps.tile([P, P], HF, tag="tpe")
                        nc.tensor.transpose(
                            tp[:], sel_sb[q][:, b2 * P:(b2 + 1) * P], ident16[:])
                        nc.scalar.activation(
                            selT[b2][:, q * P:(q + 1) * P], tp[:], AF.Copy)
                for q in range(SB):
                    ops = pe_ps.tile([P, NCLS], FP, tag="ops")
                    for b2 in range(FB):
                        nc.tensor.matmul(
                            ops[:], selT[b2][:, q * P:(q + 1) * P], pnT[b2][:],
                            start=(b2 == 0), stop=(b2 == FB - 1))
                    ot = pe_sc.tile([P, NCLS], FP, tag="ot")
                    nc.scalar.activation(ot[:], ops[:], AF.Copy, scale=sc_q[q][:, :1])
                    nc.sync.dma_start(out[q * P:(q + 1) * P, :], ot[:])

            if debug_outputs:
                nc.sync.dma_start(dbg_t[:], t_bounce[:])
                for jh in range(JC):
                    nc.sync.dma_start(
                        dbg_hnT[:, jh * 512:(jh + 1) * 512], hnT_bounce[jh][:])
                nc.sync.dma_start(dbg_h1[:], h1_bounce[:])
                nc.sync.dma_start(dbg_emb[:], emb_bounce[:])

    nc.finalize()
    return nc


# ---------------------------------------------------------------------------
# host side
# ---------------------------------------------------------------------------

def host_preprocess(inputs, n=N, ncores=NCORES, nsel=NSEL):
    R = n // ncores
    selc = nsel // ncores
    x = np.ascontiguousarray(np.asarray(inputs["x"], dtype=np.float32))
    cw = np.asarray(inputs["combine_weight"], dtype=np.float32)
    alpha = float(np.asarray(inputs["alpha"], dtype=np.float32))
    prompt = np.asarray(inputs["prompt_spec"], dtype=np.float32)
    shared = np.asarray(inputs["shared_tok"], dtype=np.float32)
    baltok = np.asarray(inputs["balance_tok"], dtype=np.float32)
    w1 = np.ascontiguousarray(np.asarray(inputs["W1"], dtype=np.float32))
    w2 = np.ascontiguousarray(np.asarray(inputs["W2"], dtype=np.float32))
    edge_index = np.asarray(inputs["edge_index"])
    labels = np.asarray(inputs["labels"])
    node_idx = np.asarray(inputs["node_idx"])

    src = edge_index[0].astype(np.int64)
    dst = edge_index[1].astype(np.int64)
    deg = (np.bincount(dst, minlength=n) + 1).astype(np.float32)
    dinv = deg ** -0.5
    wn = (dinv[src] * dinv[dst]).astype(np.float32)
    # AnormT[src, dst] += wn  (transpose of reference's Anorm[dst, src] += wn)
    anormT = np.zeros((n, n), dtype=np.float32)
    np.add.at(anormT, (src, dst), wn)
    anormT[np.arange(n), np.arange(n)] += dinv * dinv
    use_anorm = alpha > 1e-6
    use_wsym = (1.0 - alpha) > 1e-6
    post = (1.0 - alpha) if use_wsym else 1.0
    if use_anorm:
        # device expects (alpha/post_scale)-scaled adjacency; Q/P evicts
        # multiply by post_scale (see build_program)
        anormT *= alpha / post

    wcomb = (cw[0, 0] * prompt + cw[0, 1] * shared).astype(np.float32).reshape(1, -1)
    baltok2 = np.ascontiguousarray(baltok.reshape(1, -1))

    # per-node class-count matrix: selw[i, cls] = #{s: node_idx[s]=i, labels[s]=cls}
    selw_all = np.zeros((n, NCLS), dtype=np.float32)
    np.add.at(selw_all, (node_idx.astype(np.int64), labels.astype(np.int64)), 1.0)
    selw_all = selw_all.astype(np.float16)
    w116 = w1.astype(np.float16)
    w216 = w2.astype(np.float16)

    in_maps = []
    for c in range(ncores):
        sel_slice = node_idx[c * selc:(c + 1) * selc].astype(np.int32)
        sb = selc // P
        in_maps.append({
            "x_l": x[c * R:(c + 1) * R, :],
            "wcomb": wcomb,
            "baltok": baltok2,
            "anorm16": np.ascontiguousarray(
                anormT[:, c * R:(c + 1) * R]).astype(np.float16),
            "w116": w116,
            "w216": w216,
            "selidx": np.ascontiguousarray(sel_slice.reshape(sb, P).T),
            # [R, NCLS] -> [P, LB*NCLS]: row m*128+p lands at [p, m*NCLS:...]
            "selw": np.ascontiguousarray(
                selw_all[c * R:(c + 1) * R, :].reshape(R // P, P, NCLS)
                .transpose(1, 0, 2).reshape(P, (R // P) * NCLS)),
        })
    return alpha, in_maps


_prog_cache = {}


def kernel(**inputs) -> np.ndarray:
    alpha, in_maps = host_preprocess(inputs)
    key = round(alpha, 9)
    if key not in _prog_cache:
        _prog_cache[key] = build_program(alpha)
    nc = _prog_cache[key]
    res = run_bass_kernel_spmd(nc, in_maps, list(range(NCORES)))
    return np.concatenate([res.results[c]["out"] for c in range(NCORES)], axis=0)


# revision 32
# speedup vs baseline: 1.0033x; 1.0033x over previous
"""Trainium2 Bass kernel for nn_Downstream_38439957299924 (gnn_message_passing).

SPMD over 8 NeuronCores, 1D node partition (1024 rows/core).

  fea   = elu(x * wcomb)                          wcomb = cw00*prompt + cw01*shared
  agg   = Anorm @ fea                             Anorm built dense on host from edges
  h     = concat(fea, agg) * balance_tok ; hn = h / (||h|| + eps)
  sims  = hn @ hn.T ; t_i = 17th largest of row i  (K+1 = 17 with self edge)
  Wsym  = relu(sims * (sims >= min(t_i, t_j)))    == to_undirected(mean)+relu of ref
  A_tot = alpha*Anorm + (1-alpha)*Wsym
  h1    = relu((A_tot @ fea) @ W1) ; emb = (A_tot @ h1) @ W2
  out   = cos(emb[node_idx], class prototypes) / TEMP

v2 structure (vs the f32r/DRAM-spill v1):
- Everything on the fat path is fp16: anorm, fea, hnT, sims, A_tot, h1, emb.
  fp16 rhs streams 1 col/cycle through the PE (fp32 streams at half rate),
  halving matmul time; thresholds stay fp32/fp16-exact so the mask is
  bitwise consistent with the stored sims.
- simsT tiles [128 j, 1024 i] live in SBUF (64 tiles, 128KB/partition) and
  are overwritten in place by the A_totT tiles: no DRAM spill at all
  (v1 moved 96MB/core of simsT+atotT through HBM).
- relu is folded into the threshold: t' = max(t, 0), mask = sims >= min(t'_i,t'_j).
- Q = A_tot @ fea and P = A_tot @ h1 are computed TRANSPOSED (stationary =
  fea/h1 slices, moving = A_totT tiles), so QT/PT drop out in exactly the
  layout the W1/W2 matmuls need as lhsT — no interstage PE transposes.
- Per-row top-k candidates from PE-transposed copies of the fp16 sims tiles
  (max8 per 128-node window, direct from PSUM), then 3-round max/match_replace.
- DMA is quad-batched (1MB anorm/h1/fea loads).
- Collectives: AllGather fea16/hnT/t/h1/emb (all fp16) + AllReduce class sums.
"""
import numpy as np

import concourse.bacc as bacc
import concourse.bass as bass
import concourse.mybir as mybir
import concourse.tile as tile
from concourse.bass_utils import run_bass_kernel_spmd
from concourse.masks import make_identity

FP = mybir.dt.float32
HF = mybir.dt.float16
AF = mybir.ActivationFunctionType
ALU = mybir.AluOpType

NCORES = 8
N = 8192          # nodes
F = 256           # input feature dim
H2 = 512          # concat feature dim
HID = 256         # gnn hidden dim
NCLS = 10
NSEL = 4096
TEMP = 0.2
EPS = 1e-8
P = 128

NEGINF = -3.0e38


def build_program(alpha: float, n=N, ncores=NCORES, nsel=NSEL,
                  debug_outputs=False):
    """Emit the SPMD Bass program for one core (SPMD across ncores)."""
    R = n // ncores           # rows per core
    NB = n // P               # global node blocks of 128
    LB = R // P               # local node blocks
    KB = H2 // P              # feature k-blocks (4)
    FB = F // P               # 256-dim k-blocks (2)
    JC = R // 512             # 512-wide j chunks per rank
    SELC = nsel // ncores     # selected nodes per core
    SB = SELC // P            # selected blocks per core
    one_m_alpha = 1.0 - alpha
    use_anorm = alpha > 1e-6
    use_wsym = one_m_alpha > 1e-6
    # host ships anorm16 * (alpha/post_scale); Q/P evicts multiply by
    # post_scale, so the C2 merge is a plain add with no per-tile scaling.
    post_scale = one_m_alpha if use_wsym else 1.0
    # anorm16 is always needed for agg; alpha folding only when it feeds A_tot
    an_ship = (alpha / post_scale) if use_anorm else 1.0
    agg_scale = 1.0 / an_ship
    assert R % 512 == 0 and SELC % P == 0 and NB % 4 == 0

    nc = bacc.Bacc(None)

    # ---- per-core external inputs ----
    x_l = nc.declare_dram_parameter("x_l", [R, F], FP, isOutput=False)
    wcomb = nc.declare_dram_parameter("wcomb", [1, F], FP, isOutput=False)
    baltok = nc.declare_dram_parameter("baltok", [1, H2], FP, isOutput=False)
    anorm16 = nc.declare_dram_parameter("anorm16", [n, R], HF, isOutput=False)
    w116 = nc.declare_dram_parameter("w116", [F, HID], HF, isOutput=False)
    w216 = nc.declare_dram_parameter("w216", [HID, HID], HF, isOutput=False)
    selidx = nc.declare_dram_parameter("selidx", [P, SB], mybir.dt.int32, isOutput=False)
    # selw pre-transposed on host to [P, LB*NCLS] (partition-major)
    selw = nc.declare_dram_parameter("selw", [P, LB * NCLS], HF, isOutput=False)
    out = nc.declare_dram_parameter("out", [SELC, NCLS], FP, isOutput=True)
    if debug_outputs:
        dbg_t = nc.declare_dram_parameter("dbg_t", [R, 1], HF, isOutput=True)
        dbg_agg = nc.declare_dram_parameter("dbg_agg", [F, R], FP, isOutput=True)
        dbg_h1 = nc.declare_dram_parameter("dbg_h1", [R, HID], HF, isOutput=True)
        dbg_emb = nc.declare_dram_parameter("dbg_emb", [R, HID], HF, isOutput=True)
        dbg_hnT = nc.declare_dram_parameter("dbg_hnT", [H2, R], HF, isOutput=True)

    # ---- internal DRAM ----
    fea16_bounce = nc.dram_tensor("fea16_bounce", [R, F], HF)
    fea16_full = nc.dram_tensor("fea16_full", [n, F], HF, addr_space="Shared")
    # hnT split into JC column-halves so each half's AllGather can fire as
    # soon as the corresponding local i-blocks are normalized
    hnT_bounce = [nc.dram_tensor(f"hnT_bounce{j}", [H2, 512], HF)
                  for j in range(JC)]
    hnT_all = [nc.dram_tensor(f"hnT_all{j}", [ncores * H2, 512], HF,
                              addr_space="Shared") for j in range(JC)]
    t_bounce = nc.dram_tensor("t_bounce", [R, 1], HF)
    t_full = nc.dram_tensor("t_full", [n, 1], HF, addr_space="Shared")
    h1_bounce = nc.dram_tensor("h1_bounce", [R, HID], HF)
    h1_full = nc.dram_tensor("h1_full", [n, HID], HF, addr_space="Shared")
    emb_bounce = nc.dram_tensor("emb_bounce", [R, HID], HF)
    emb_full = nc.dram_tensor("emb_full", [n, HID], HF, addr_space="Shared")
    sums_bounce = nc.dram_tensor("sums_bounce", [HID, NCLS], FP)
    sums_red = nc.dram_tensor("sums_red", [HID, NCLS], FP, addr_space="Shared")

    rg = [list(range(ncores))]

    def ag(in_ap, out_ap):
        nc.gpsimd.collective_compute(
            "AllGather", ALU.bypass, replica_groups=rg, ins=[in_ap], outs=[out_ap])

    with tile.TileContext(nc) as tc:
        with (
            tc.tile_pool(name="const", bufs=1) as const,
            tc.tile_pool(name="live", bufs=1) as live,
        ):
            ident32 = const.tile([P, P], FP)
            make_identity(nc, ident32)
            ident16 = const.tile([P, P], HF)
            nc.scalar.activation(ident16[:], ident32[:], AF.Copy)
            hnT_l = [live.tile([P, R], HF, tag=f"hnT{k}", name=f"hnT{k}")
                     for k in range(KB)]
            emball = live.tile([P, LB * HID], HF, tag="emball", name="emball")
            # the big one: simsT (later overwritten in place by A_totT)
            sims_sb = [live.tile([P, R], HF, tag=f"sims{m}", name=f"sims{m}")
                       for m in range(NB)]

            # ===== phase 0: fea_l = elu(x_l * wcomb); all-gather fea16 =====
            with (
                tc.tile_pool(name="pfea", bufs=1) as pfea,
                tc.tile_pool(name="p0c", bufs=1) as p0c,
                tc.tile_pool(name="p0", bufs=3) as p0,
            ):
                fea_l = [pfea.tile([P, F], FP, tag=f"fea{b}", name=f"fea{b}")
                         for b in range(LB)]
                wcomb_b = p0c.tile([P, F], FP)
                nc.sync.dma_start(wcomb_b[:], wcomb[:1, :].to_broadcast([P, F]))
                baltok_b = p0c.tile([P, H2], FP)
                nc.sync.dma_start(baltok_b[:], baltok[:1, :].to_broadcast([P, H2]))
                for b in range(LB):
                    xt = p0.tile([P, F], FP, tag="xt")
                    nc.sync.dma_start(xt[:], x_l[b * P:(b + 1) * P, :])
                    z = p0.tile([P, F], FP, tag="z")
                    nc.vector.tensor_mul(z[:], xt[:], wcomb_b[:])
                    r = p0.tile([P, F], FP, tag="r")
                    nc.scalar.activation(r[:], z[:], AF.Relu)
                    m0 = p0.tile([P, F], FP, tag="m0")
                    nc.vector.tensor_scalar_min(m0[:], z[:], 0.0)
                    e = p0.tile([P, F], FP, tag="e")
                    nc.scalar.activation(e[:], m0[:], AF.Exp)
                    nc.vector.scalar_tensor_tensor(
                        fea_l[b][:], e[:], -1.0, r[:], op0=ALU.add, op1=ALU.add)
                    f16 = p0.tile([P, F], HF, tag="f16")
                    nc.scalar.activation(f16[:], fea_l[b][:], AF.Copy)
                    nc.sync.dma_start(fea16_bounce[b * P:(b + 1) * P, :], f16[:])
                ag(fea16_bounce[:], fea16_full[:])

                # ===== phase A: aggT[f, i] = sum_j fea16[j, f] * anorm16[j, i] =====
                with tc.tile_pool(name="pagg", bufs=1) as pagg:
                    aggT_sb = [pagg.tile([P, R], FP, tag=f"aggT{mf}", name=f"aggT{mf}")
                               for mf in range(FB)]
                    with (
                        tc.tile_pool(name="pa_in", bufs=3) as pa_in,
                        tc.tile_pool(name="pa_ps", bufs=1, space="PSUM") as pa_ps,
                    ):
                        agg_ps = [[pa_ps.tile([P, 512], FP, tag=f"aps{mf}_{c2}",
                                              name=f"aps{mf}_{c2}")
                                   for c2 in range(JC)] for mf in range(FB)]
                        for kq in range(NB // 4):
                            fk4 = pa_in.tile([P, 4 * F], HF, tag="fk4")
                            nc.scalar.dma_start(
                                fk4[:].rearrange("p (four f) -> p four f", four=4),
                                fea16_full[kq * 4 * P:(kq + 1) * 4 * P, :].rearrange(
                                    "(four p) f -> p four f", four=4))
                            an4 = pa_in.tile([P, 4 * R], HF, tag="an4")
                            nc.sync.dma_start(
                                an4[:].rearrange("p (four i) -> p four i", four=4),
                                anorm16[kq * 4 * P:(kq + 1) * 4 * P, :].rearrange(
                                    "(four p) i -> p four i", four=4))
                            for q4 in range(4):
                                kj = kq * 4 + q4
                                for mf in range(FB):
                                    for c2 in range(JC):
                                        nc.tensor.matmul(
                                            agg_ps[mf][c2][:],
                                            fk4[:, q4 * F + mf * P:q4 * F + (mf + 1) * P],
                                            an4[:, q4 * R + c2 * 512:q4 * R + (c2 + 1) * 512],
                                            start=(kj == 0), stop=(kj == NB - 1))
                        for mf in range(FB):
                            for c2 in range(JC):
                                nc.scalar.activation(
                                    aggT_sb[mf][:, c2 * 512:(c2 + 1) * 512],
                                    agg_ps[mf][c2][:], AF.Copy, scale=agg_scale)
                        if debug_outputs:
                            for mf in range(FB):
                                nc.sync.dma_start(
                                    dbg_agg[mf * P:(mf + 1) * P, :], aggT_sb[mf][:])

                    # ===== phase A2: h = concat(fea, aggT.T)*baltok; hn; hnT =====
                    with (
                        tc.tile_pool(name="ph_ps", bufs=2, space="PSUM") as ph_ps,
                        tc.tile_pool(name="ph_sb", bufs=2) as ph_sb,
                    ):
                        for b in range(LB):
                            h = ph_sb.tile([P, H2], FP, tag="h")
                            nc.vector.tensor_mul(h[:, :F], fea_l[b][:], baltok_b[:, :F])
                            for mf in range(FB):
                                tp = ph_ps.tile([P, P], FP, tag="tp")
                                nc.tensor.transpose(
                                    tp[:], aggT_sb[mf][:, b * P:(b + 1) * P], ident32[:])
                                nc.scalar.activation(
                                    h[:, F + mf * P:F + (mf + 1) * P], tp[:], AF.Copy)
                            nc.vector.tensor_mul(h[:, F:], h[:, F:], baltok_b[:, F:])
                            sq = ph_sb.tile([P, H2], FP, tag="sq")
                            ssq = ph_sb.tile([P, 1], FP, tag="ssq")
                            nc.scalar.activation(sq[:], h[:], AF.Square, accum_out=ssq[:])
                            nrm = ph_sb.tile([P, 1], FP, tag="nrm")
                            nc.scalar.activation(nrm[:], ssq[:], AF.Sqrt)
                            nc.vector.tensor_scalar_add(nrm[:], nrm[:], EPS)
                            inv = ph_sb.tile([P, 1], FP, tag="inv")
                            nc.vector.reciprocal(inv[:], nrm[:])
                            hn = ph_sb.tile([P, H2], FP, tag="hn")
                            nc.vector.tensor_scalar(hn[:], h[:], inv[:, :1], None, ALU.mult)
                            for kk in range(KB):
                                tp2 = ph_ps.tile([P, P], FP, tag="tp2")
                                nc.tensor.transpose(
                                    tp2[:], hn[:, kk * P:(kk + 1) * P], ident32[:])
                                nc.scalar.activation(
                                    hnT_l[kk][:, b * P:(b + 1) * P], tp2[:], AF.Copy)
                            # fire each 512-column half's gather as soon as done
                            if (b + 1) % 4 == 0:
                                jh = b // 4
                                for kk in range(KB):
                                    nc.sync.dma_start(
                                        hnT_bounce[jh][kk * P:(kk + 1) * P, :],
                                        hnT_l[kk][:, jh * 512:(jh + 1) * 512])
                                ag(hnT_bounce[jh][:], hnT_all[jh][:])

            # ===== phase C1: simsT tiles in SBUF (fp16); candidates from
            #       PE-transposed tiles batched 4-wide in PSUM (max8 per
            #       512-node window) =====
            # C2's input pools are opened BEFORE C1 so the anorm/fea prefetch
            # DMAs overlap C1's tail instead of waiting on the pool barrier
            pc_tib_cm = tc.tile_pool(name="pc_tib", bufs=1)
            pc_tib = pc_tib_cm.__enter__()
            pc_in_cm = tc.tile_pool(name="pc_in", bufs=2)
            pc_in = pc_in_cm.__enter__()
            with tc.tile_pool(name="pb_cand", bufs=1) as pb_cand:
                cand = [pb_cand.tile([P, 2 * NB], FP, tag=f"cand{m8}", name=f"cand{m8}")
                        for m8 in range(LB)]
                with (
                    tc.tile_pool(name="pc1_in", bufs=3) as pc1_in,
                    tc.tile_pool(name="pc1_ps", bufs=2, space="PSUM") as pc1_ps,
                    tc.tile_pool(name="pc1_tp", bufs=1, space="PSUM") as pc1_tp,
                ):
                    # tp_acc: per local i-block, a [128, 512] fp16 window
                    # accumulating 4 mj transposes; two i-blocks share a bank
                    tp_acc = [pc1_tp.tile([P, 1024], HF, tag=f"tpa{q}",
                                          name=f"tpa{q}") for q in range(LB // 2)]
                    for jc in range(JC):
                        for r0 in range(ncores):
                            lhs = pc1_in.tile([P, KB * 512], HF, tag="lhs")
                            nc.sync.dma_start(
                                lhs[:].rearrange("p (kk j) -> p kk j", kk=KB),
                                hnT_all[jc][r0 * H2:(r0 + 1) * H2, :].rearrange(
                                    "(kk p) j -> p kk j", kk=KB))
                            mq = r0 * JC + jc
                            for sub in range(4):
                                mj = mq * 4 + sub
                                for hh in range(JC):
                                    ps = pc1_ps.tile([P, 512], FP, tag="ps")
                                    for kk in range(KB):
                                        nc.tensor.matmul(
                                            ps[:],
                                            lhs[:, kk * 512 + sub * P:kk * 512 + (sub + 1) * P],
                                            hnT_l[kk][:, hh * 512:(hh + 1) * 512],
                                            start=(kk == 0), stop=(kk == KB - 1))
                                    nc.scalar.activation(
                                        sims_sb[mj][:, hh * 512:(hh + 1) * 512],
                                        ps[:], AF.Copy)
                                for ib8 in range(LB):
                                    nc.tensor.transpose(
                                        tp_acc[ib8 // 2][
                                            :, (ib8 % 2) * 512 + sub * P:
                                            (ib8 % 2) * 512 + (sub + 1) * P],
                                        sims_sb[mj][:, ib8 * P:(ib8 + 1) * P],
                                        ident16[:])
                            for ib8 in range(LB):
                                nc.vector.max(
                                    cand[ib8][:, mq * 8:(mq + 1) * 8],
                                    tp_acc[ib8 // 2][
                                        :, (ib8 % 2) * 512:(ib8 % 2 + 1) * 512])
                # merge candidates -> t' = max(17th largest, 0) per local row
                with tc.tile_pool(name="pbm_sb", bufs=2) as pbm_sb:
                    for m8 in range(LB):
                        t8a = pbm_sb.tile([P, 8], FP, tag="t8a")
                        nc.vector.max(t8a[:], cand[m8][:])
                        nc.vector.match_replace(
                            cand[m8][:], t8a[:], cand[m8][:], NEGINF)
                        t8b = pbm_sb.tile([P, 8], FP, tag="t8b")
                        nc.vector.max(t8b[:], cand[m8][:])
                        nc.vector.match_replace(
                            cand[m8][:], t8b[:], cand[m8][:], NEGINF)
                        t8c = pbm_sb.tile([P, 8], FP, tag="t8c")
                        nc.vector.max(t8c[:], cand[m8][:])
                        tcl = pbm_sb.tile([P, 1], HF, tag="tcl")
                        nc.vector.tensor_scalar(
                            tcl[:], t8c[:, :1], 0.0, None, ALU.max)
                        nc.sync.dma_start(
                            t_bounce[m8 * P:(m8 + 1) * P, :], tcl[:])
            ag(t_bounce[:], t_full[:])

            # ===== phase C2: A_totT = alpha*anorm + (1-alpha)*masked sims,
            #       written in place over sims_sb; fused QT = (A_tot @ fea).T =====
            with (
                tc.tile_pool(name="pqt", bufs=1) as pqt,
                tc.tile_pool(name="pc_qps", bufs=1, space="PSUM") as pc_qps,
                tc.tile_pool(name="pc_sb", bufs=3) as pc_sb,
            ):
                qT16 = [pqt.tile([P, R], HF, tag=f"qT{fs}", name=f"qT{fs}")
                        for fs in range(FB)]
                t_i16 = pc_tib.tile([P, R], HF)
                nc.sync.dma_start(
                    t_i16[:],
                    t_bounce.rearrange("a b -> b a")[:1, :].to_broadcast([P, R]))
                # t_full as [128, NB]: column mj = t' for node block mj. Loaded
                # block-per-partition (contiguous 256B lines) then PE-transposed
                # — the direct [p, m] gather would be a 2-byte-strided DMA.
                # tfneg = -(t_j - half ulp16): the Sign-activation bias, shifted
                # off the fp16 grid so sign(st + tfneg) > 0  <=>  st >= t_j.
                tfL = pc_tib.tile([NB, P], HF)
                nc.sync.dma_start(
                    tfL[:NB, :], t_full.rearrange("(m p) one -> m (p one)", p=P))
                tf_sb = pc_tib.tile([P, NB], HF)
                with tc.tile_pool(name="ptf", bufs=1, space="PSUM") as ptf:
                    tfp = ptf.tile([P, NB], HF)
                    nc.tensor.transpose(
                        tfp[:, :], tfL[:NB, :], ident16[:NB, :NB])
                    nc.scalar.activation(tf_sb[:], tfp[:], AF.Copy)
                tfneg = pc_tib.tile([P, NB], FP)
                nc.vector.tensor_scalar(
                    tfneg[:], tf_sb[:], -(1.0 - 2.0 ** -12), 1e-8,
                    ALU.mult, ALU.add)
                bias30 = pc_tib.tile([P, 1], FP)
                nc.vector.memset(bias30[:], 30.0)
                q_ps = [[pc_qps.tile([P, 512], FP, tag=f"q{fs}_{ih}",
                                     name=f"q{fs}_{ih}")
                         for ih in range(JC)] for fs in range(FB)]
                for mq in range(NB // 4):
                    an4 = pc_in.tile([P, 4 * R], HF, tag="an4")
                    if use_anorm:
                        nc.sync.dma_start(
                            an4[:].rearrange("p (four i) -> p four i", four=4),
                            anorm16[mq * 4 * P:(mq + 1) * 4 * P, :].rearrange(
                                "(four p) i -> p four i", four=4))
                    fq4 = pc_in.tile([P, 4 * F], HF, tag="fq4")
                    nc.scalar.dma_start(
                        fq4[:].rearrange("p (four f) -> p four f", four=4),
                        fea16_full[mq * 4 * P:(mq + 1) * 4 * P, :].rearrange(
                            "(four p) f -> p four f", four=4))
                    for q4 in range(4):
                        mj = mq * 4 + q4
                        st = sims_sb[mj]
                        if use_wsym:
                            # mask = (st >= t_i) OR (st >= t_j), thresholds
                            # clamped >= 0 (relu folded in). is_ge has no fast
                            # DVE path, so the mask is built comparison-free:
                            #   sj = sign(st - t_j')            in {-1,+1}
                            #   q  = max(st - t_i, sj)          >= 0 iff mask
                            #   g  = sigmoid(1e9*q + 30)        exactly {0,1}
                            # (q is an exact-sign fp16: nonzero |q| >= 2^-24,
                            # so 1e9*q +- 30 saturates the sigmoid both ways)
                            sj = pc_sb.tile([P, R], HF, tag="sj")
                            nc.scalar.activation(
                                sj[:], st[:], AF.Sign, bias=tfneg[:, mj:mj + 1])
                            d = pc_sb.tile([P, R], HF, tag="d")
                            nc.vector.tensor_tensor(
                                d[:], st[:], t_i16[:], ALU.subtract)
                            q = pc_sb.tile([P, R], HF, tag="q")
                            nc.vector.tensor_tensor(d[:], d[:], sj[:], ALU.max)
                            nc.scalar.activation(
                                q[:], d[:], AF.Sigmoid, bias=bias30[:, :1],
                                scale=1e9)
                            eng = nc.gpsimd if mj % 3 == 1 else nc.vector
                            if use_anorm:
                                w2 = pc_sb.tile([P, R], HF, tag="w2")
                                eng.tensor_tensor(w2[:], st[:], q[:], ALU.mult)
                                eng.tensor_tensor(
                                    st[:], w2[:], an4[:, q4 * R:(q4 + 1) * R],
                                    ALU.add)
                            else:
                                eng.tensor_tensor(st[:], st[:], q[:], ALU.mult)
                        else:
                            nc.vector.tensor_copy(
                                st[:], an4[:, q4 * R:(q4 + 1) * R])
                        # QT[f, i] += fea16[j, f].T @ at[j, i]
                        for fs in range(FB):
                            for ih in range(JC):
                                nc.tensor.matmul(
                                    q_ps[fs][ih][:],
                                    fq4[:, q4 * F + fs * P:q4 * F + (fs + 1) * P],
                                    st[:, ih * 512:(ih + 1) * 512],
                                    start=(mj == 0), stop=(mj == NB - 1))
                for fs in range(FB):
                    for ih in range(JC):
                        nc.scalar.activation(
                            qT16[fs][:, ih * 512:(ih + 1) * 512],
                            q_ps[fs][ih][:], AF.Copy, scale=post_scale)

                # ===== phase D1: h1 = relu(QT.T @ W1); all-gather h1 =====
                with (
                    tc.tile_pool(name="pd1_ps", bufs=2, space="PSUM") as pd1_ps,
                    tc.tile_pool(name="pd1_sb", bufs=2) as pd1_sb,
                    tc.tile_pool(name="pd1_w", bufs=1) as pd1_w,
                ):
                    w1_sb = [pd1_w.tile([P, HID], HF, tag=f"w1_{k2}", name=f"w1_{k2}")
                             for k2 in range(FB)]
                    for k2 in range(FB):
                        nc.sync.dma_start(w1_sb[k2][:], w116[k2 * P:(k2 + 1) * P, :])
                    h1all = pd1_sb.tile([P, LB * HID], HF)
                    for m8 in range(LB):
                        ps = pd1_ps.tile([P, HID], FP, tag="psh")
                        for k2 in range(FB):
                            nc.tensor.matmul(
                                ps[:], qT16[k2][:, m8 * P:(m8 + 1) * P], w1_sb[k2][:],
                                start=(k2 == 0), stop=(k2 == FB - 1))
                        nc.scalar.activation(
                            h1all[:, m8 * HID:(m8 + 1) * HID], ps[:], AF.Relu)
                    nc.sync.dma_start(
                        h1_bounce.rearrange("(m p) e -> p m e", p=P),
                        h1all[:].rearrange("p (m e) -> p m e", m=LB))
            pc_in_cm.__exit__(None, None, None)
            pc_tib_cm.__exit__(None, None, None)
            ag(h1_bounce[:], h1_full[:])

            # ===== phase D2: PT = (A_tot @ h1_full).T =====
            with (
                tc.tile_pool(name="ppt", bufs=1) as ppt,
                tc.tile_pool(name="pd2_in", bufs=3) as pd2_in,
                tc.tile_pool(name="pd2_ps", bufs=1, space="PSUM") as pd2_ps,
            ):
                pT16 = [ppt.tile([P, R], HF, tag=f"pT{hs}", name=f"pT{hs}")
                        for hs in range(FB)]
                p_ps = [[pd2_ps.tile([P, 512], FP, tag=f"p{hs}_{ih}",
                                     name=f"p{hs}_{ih}")
                         for ih in range(JC)] for hs in range(FB)]
                for kq in range(NB // 4):
                    h1k4 = pd2_in.tile([P, 4 * HID], HF, tag="h1k4")
                    nc.sync.dma_start(
                        h1k4[:].rearrange("p (four e) -> p four e", four=4),
                        h1_full[kq * 4 * P:(kq + 1) * 4 * P, :].rearrange(
                            "(four p) e -> p four e", four=4))
                    for q4 in range(4):
                        kj = kq * 4 + q4
                        for hs in range(FB):
                            for ih in range(JC):
                                nc.tensor.matmul(
                                    p_ps[hs][ih][:],
                                    h1k4[:, q4 * HID + hs * P:q4 * HID + (hs + 1) * P],
                                    sims_sb[kj][:, ih * 512:(ih + 1) * 512],
                                    start=(kj == 0), stop=(kj == NB - 1))
                for hs in range(FB):
                    for ih in range(JC):
                        nc.scalar.activation(
                            pT16[hs][:, ih * 512:(ih + 1) * 512],
                            p_ps[hs][ih][:], AF.Copy, scale=post_scale)

                # ===== phase D3: emb = PT.T @ W2; all-gather emb =====
                with (
                    tc.tile_pool(name="pd3_ps", bufs=2, space="PSUM") as pd3_ps,
                    tc.tile_pool(name="pd3_w", bufs=1) as pd3_w,
                    tc.tile_pool(name="pd3_sb", bufs=2) as pd3_sb,
                ):
                    w2_sb = [pd3_w.tile([P, HID], HF, tag=f"w2_{k2}", name=f"w2_{k2}")
                             for k2 in range(FB)]
                    for k2 in range(FB):
                        nc.sync.dma_start(w2_sb[k2][:], w216[k2 * P:(k2 + 1) * P, :])
                    for m8 in range(LB):
                        ps = pd3_ps.tile([P, HID], FP, tag="pse")
                        for k2 in range(FB):
                            nc.tensor.matmul(
                                ps[:], pT16[k2][:, m8 * P:(m8 + 1) * P], w2_sb[k2][:],
                                start=(k2 == 0), stop=(k2 == FB - 1))
                        nc.scalar.activation(
                            emball[:, m8 * HID:(m8 + 1) * HID], ps[:], AF.Copy)
                    nc.sync.dma_start(
                        emb_bounce.rearrange("(m p) e -> p m e", p=P),
                        emball[:].rearrange("p (m e) -> p m e", m=LB))
            ag(emb_bounce[:], emb_full[:])

            # ===== phase E: prototypes + cosine scores =====
            with (
                tc.tile_pool(name="pe_sb", bufs=1) as pe_sb,
                tc.tile_pool(name="pe_ps", bufs=1, space="PSUM") as pe_ps,
                tc.tile_pool(name="pe_sc", bufs=2) as pe_sc,
            ):
                # class sums from LOCAL emb rows via the host-built per-node
                # count matrix selw (runs concurrently with the emb AllGather)
                selw_sb = pe_sb.tile([P, LB * NCLS], HF)
                nc.sync.dma_start(selw_sb[:], selw[:])
                sums_ps = [pe_ps.tile([P, NCLS], FP, tag=f"sums{b2}", name=f"sums{b2}")
                           for b2 in range(FB)]
                for m8 in range(LB):
                    for b2 in range(FB):
                        nc.tensor.matmul(
                            sums_ps[b2][:],
                            emball[:, m8 * HID + b2 * P:m8 * HID + (b2 + 1) * P],
                            selw_sb[:, m8 * NCLS:(m8 + 1) * NCLS],
                            start=(m8 == 0), stop=(m8 == LB - 1))
                for b2 in range(FB):
                    st = pe_sc.tile([P, NCLS], FP, tag="st")
                    nc.scalar.activation(st[:], sums_ps[b2][:], AF.Copy)
                    nc.sync.dma_start(sums_bounce[b2 * P:(b2 + 1) * P, :], st[:])
                nc.gpsimd.collective_compute(
                    "AllReduce", ALU.add, replica_groups=rg,
                    ins=[sums_bounce[:]], outs=[sums_red[:]])
                idx_sb = pe_sb.tile([P, SB], mybir.dt.int32)
                nc.sync.dma_start(idx_sb[:], selidx[:])
                sel_sb = [pe_sb.tile([P, HID], HF, tag=f"sel{q}", name=f"sel{q}")
                          for q in range(SB)]
                sc_q = [pe_sb.tile([P, 1], FP, tag=f"scq{q}", name=f"scq{q}")
                        for q in range(SB)]
                for q in range(SB):
                    nc.gpsimd.indirect_dma_start(
                        out=sel_sb[q][:], out_offset=None,
                        in_=emb_full[:],
                        in_offset=bass.IndirectOffsetOnAxis(
                            ap=idx_sb[:, q:q + 1], axis=0))
                    sq = pe_sc.tile([P, HID], FP, tag="sq")
                    ssq = pe_sc.tile([P, 1], FP, tag="ssq")
                    nc.scalar.activation(
                        sq[:], sel_sb[q][:], AF.Square, accum_out=ssq[:])
                    nrm = pe_sc.tile([P, 1], FP, tag="nrm2")
                    nc.scalar.activation(nrm[:], ssq[:], AF.Sqrt)
                    nc.vector.tensor_scalar_add(nrm[:], nrm[:], EPS)
                    nc.vector.tensor_scalar_mul(nrm[:], nrm[:], TEMP)
                    nc.vector.reciprocal(sc_q[q][:], nrm[:])
                sums_sb = [pe_sb.tile([P, NCLS], FP, tag=f"smr{b2}", name=f"smr{b2}")
                           for b2 in range(FB)]
                ones_col = pe_sb.tile([P, 1], FP)
                nc.vector.memset(ones_col[:], 1.0)
                ones_row = pe_sb.tile([1, P], FP)
                nc.vector.memset(ones_row[:1, :], 1.0)
                nps = pe_ps.tile([1, NCLS], FP, tag="nps")
                for b2 in range(FB):
                    nc.sync.dma_start(sums_sb[b2][:], sums_red[b2 * P:(b2 + 1) * P, :])
                    sqs = pe_sc.tile([P, NCLS], FP, tag="sqs")
                    nc.scalar.activation(sqs[:], sums_sb[b2][:], AF.Square)
                    nc.tensor.matmul(nps[:1, :], ones_col[:, :1], sqs[:],
                                     start=(b2 == 0), stop=(b2 == FB - 1))
                nrmc = pe_sc.tile([1, NCLS], FP, tag="nrmc")
                nc.scalar.activation(nrmc[:1, :], nps[:1, :], AF.Sqrt)
                nc.vector.tensor_scalar_add(nrmc[:1, :], nrmc[:1, :], EPS)
                invc = pe_sc.tile([1, NCLS], FP, tag="invc")
                nc.vector.reciprocal(invc[:1, :], nrmc[:1, :])
                bcp = pe_ps.tile([P, NCLS], FP, tag="bcp")
                nc.tensor.matmul(bcp[:], ones_row[:1, :], invc[:1, :],
                                 start=True, stop=True)
                bc_sb = pe_sb.tile([P, NCLS], FP)
                nc.scalar.activation(bc_sb[:], bcp[:], AF.Copy)
                pnT = [pe_sb.tile([P, NCLS], FP, tag=f"pnT{b2}", name=f"pnT{b2}")
                       for b2 in range(FB)]
                for b2 in range(FB):
                    nc.vector.tensor_mul(pnT[b2][:], sums_sb[b2][:], bc_sb[:])
                selT = [pe_sb.tile([P, SELC], FP, tag=f"selT{b2}", name=f"selT{b2}")
                        for b2 in range(FB)]
                for q in range(SB):
                    for b2 in range(FB):
                        tp = pe_ps.tile([P, P], HF, tag="tpe")
                        nc.tensor.transpose(
                            tp[:], sel_sb[q][:, b2 * P:(b2 + 1) * P], ident16[:])
                        nc.scalar.activation(
                            selT[b2][:, q * P:(q + 1) * P], tp[:], AF.Copy)
                for q in range(SB):
                    ops = pe_ps.tile([P, NCLS], FP, tag="ops")
                    for b2 in range(FB):
                        nc.tensor.matmul(
                            ops[:], selT[b2][:, q * P:(q + 1) * P], pnT[b2][:],
                            start=(b2 == 0), stop=(b2 == FB - 1))
                    ot = pe_sc.tile([P, NCLS], FP, tag="ot")
                    nc.scalar.activation(ot[:], ops[:], AF.Copy, scale=sc_q[q][:, :1])
                    nc.sync.dma_start(out[q * P:(q + 1) * P, :], ot[:])

            if debug_outputs:
                nc.sync.dma_start(dbg_t[:], t_bounce[:])
                for jh in range(JC):
                    nc.sync.dma_start(
                        dbg_hnT[:, jh * 512:(jh + 1) * 512], hnT_bounce[jh][:])
                nc.sync.dma_start(dbg_h1[:], h1_bounce[:])
                nc.sync.dma_start(dbg_emb[:], emb_bounce[:])

    nc.finalize()
    return nc


# ---------------------------------------------------------------------------
# host side
# ---------------------------------------------------------------------------

def host_preprocess(inputs, n=N, ncores=NCORES, nsel=NSEL):
    R = n // ncores
    selc = nsel // ncores
    x = np.ascontiguousarray(np.asarray(inputs["x"], dtype=np.float32))
    cw = np.asarray(inputs["combine_weight"], dtype=np.float32)
    alpha = float(np.asarray(inputs["alpha"], dtype=np.float32))
    prompt = np.asarray(inputs["prompt_spec"], dtype=np.float32)
    shared = np.asarray(inputs["shared_tok"], dtype=np.float32)
    baltok = np.asarray(inputs["balance_tok"], dtype=np.float32)
    w1 = np.ascontiguousarray(np.asarray(inputs["W1"], dtype=np.float32))
    w2 = np.ascontiguousarray(np.asarray(inputs["W2"], dtype=np.float32))
    edge_index = np.asarray(inputs["edge_index"])
    labels = np.asarray(inputs["labels"])
    node_idx = np.asarray(inputs["node_idx"])

    src = edge_index[0].astype(np.int64)
    dst = edge_index[1].astype(np.int64)
    deg = (np.bincount(dst, minlength=n) + 1).astype(np.float32)
    dinv = deg ** -0.5
    wn = (dinv[src] * dinv[dst]).astype(np.float32)
    # AnormT[src, dst] += wn  (transpose of reference's Anorm[dst, src] += wn)
    anormT = np.zeros((n, n), dtype=np.float32)
    np.add.at(anormT, (src, dst), wn)
    anormT[np.arange(n), np.arange(n)] += dinv * dinv
    use_anorm = alpha > 1e-6
    use_wsym = (1.0 - alpha) > 1e-6
    post = (1.0 - alpha) if use_wsym else 1.0
    if use_anorm:
        # device expects (alpha/post_scale)-scaled adjacency; Q/P evicts
        # multiply by post_scale (see build_program)
        anormT *= alpha / post

    wcomb = (cw[0, 0] * prompt + cw[0, 1] * shared).astype(np.float32).reshape(1, -1)
    baltok2 = np.ascontiguousarray(baltok.reshape(1, -1))

    # per-node class-count matrix: selw[i, cls] = #{s: node_idx[s]=i, labels[s]=cls}
    selw_all = np.zeros((n, NCLS), dtype=np.float32)
    np.add.at(selw_all, (node_idx.astype(np.int64), labels.astype(np.int64)), 1.0)
    selw_all = selw_all.astype(np.float16)
    w116 = w1.astype(np.float16)
    w216 = w2.astype(np.float16)

    in_maps = []
    for c in range(ncores):
        sel_slice = node_idx[c * selc:(c + 1) * selc].astype(np.int32)
        sb = selc // P
        in_maps.append({
            "x_l": x[c * R:(c + 1) * R, :],
            "wcomb": wcomb,
            "baltok": baltok2,
            "anorm16": np.ascontiguousarray(
                anormT[:, c * R:(c + 1) * R]).astype(np.float16),
            "w116": w116,
            "w216": w216,
            "selidx": np.ascontiguousarray(sel_slice.reshape(sb, P).T),
            # [R, NCLS] -> [P, LB*NCLS]: row m*128+p lands at [p, m*NCLS:...]
            "selw": np.ascontiguousarray(
                selw_all[c * R:(c + 1) * R, :].reshape(R // P, P, NCLS)
                .transpose(1, 0, 2).reshape(P, (R // P) * NCLS)),
        })
    return alpha, in_maps


_prog_cache = {}


def kernel(**inputs) -> np.ndarray:
    alpha, in_maps = host_preprocess(inputs)
    key = round(alpha, 9)
    if key not in _prog_cache:
        _prog_cache[key] = build_program(alpha)
    nc = _prog_cache[key]
    res = run_bass_kernel_spmd(nc, in_maps, list(range(NCORES)))
    return np.concatenate([res.results[c]["out"] for c in range(NCORES)], axis=0)
